# revision 20
# baseline (speedup 1.0000x reference)
"""Causal single-head attention (b=4, n=2048, d=1024) on 8 trn2 cores.

Sharding: 2 cores per batch element; even-parity cores take even-index
q-blocks (odd causal limit), odd-parity cores take odd-index ones, so
every core processes one 128-row q-block at each capacity in
{2,4,...,16} key-blocks (72 key-block visits/core, pure SPMD — the
instruction stream is identical on all cores, only data differs).

Algebraic restructure vs the direct form (out = softmax(xWq (xWk)^T
/ 32) x Wv), using associativity on BOTH sides of the softmax:

  scores^T = xk G^T xq^T       with G = Wq Wk^T / 32  (host, shared)
  out      = (W xk) Wv         with W the softmax weights

so the device never projects K or V over the 2048 keys at all. Per
core: PT = G^T xq^T over its own 1024 q rows (27us, not duplicated
across the pair), scores S^T[k,q] = xkT . PT with raw xkT chunks as
the matmul stationary (k lands on partitions, which is exactly what
the weight-application matmul wants — no PE transposes of softmax
weights), T[q,d] = sum_k exp[k,q] xk[k,d] accumulated per q-block
(the exp tiles are the stationary, so softmax row-sums ride along as
1-cycle ones-matmuls), and finally out = (T/rowsum) Wv — one 128x1024
x 1024x1024 GEMM per q-block (27us total, replacing the 55us
duplicated V projection). The 1/rowsum folds into the T PSUM->SBUF
cast for free; T^T for the final GEMM needs 8 PE transposes per slot.

Softmax skips the max-subtraction (scores/32 are ~N(0,1); exp stays
far inside f32 range), so exp is a single PSUM->SBUF ACT op.

Attention runs kb-major over slot PAIRS so each key-block's stationary
LDWEIGHTS is amortized over both active q-blocks (moving dim 256).
PSUM zero regions are 2KB (a bank) and admit one accumulation group at
a time: per pair 2 T banks/slot + 1 sums bank/slot + 2 score banks =
all 8 banks; the epilogue reuses freed T/score banks.

Everything lives in bf16 on SBUF (f32 PSUM accumulate): halves DMA and
SBUF footprint, LDWEIGHTS at 1.0 cyc/row (hidden under 512-wide
matmuls), and narrow matmuls run at full rate (f32r would be 4x
penalized below 256-wide outputs). Input DMAs are spread across the
sync/gpsimd/vector/scalar queues so the startup-critical tensors
arrive in parallel.
"""

import numpy as np

P = 128
B, N, D = 4, 2048, 1024
NCORES = 8
CAPS = (2, 4, 6, 8, 10, 12, 14, 16)  # key-block capacity per slot
PAIRS = ((6, 7), (4, 5), (2, 3), (0, 1))  # big pair first, small at tail
NEG = -1.0e30
DC = D // P  # 8 contraction chunks
NKB = N // P  # 16 key blocks

MM_DT = "bf16"  # compat knob for test.py; bf16 is the only path now

_prog_cache = {}


def _split_multi_waits(nc, max_waits=1):
    """walrus in this container rejects more than one sem wait per
    instruction ("Too many sync wait commands"). After Tile scheduling,
    hoist extra waits onto same-engine nops inserted just before the
    instruction (same blocking semantics: engine queues are in-order)."""
    from concourse import mybir

    n = 0
    for fn in nc.m.functions:
        for bb in fn.blocks:
            out = []
            for ins in bb.instructions:
                si = ins.sync_info
                waits = list(si.on_wait) if si and si.on_wait else []
                if len(waits) > max_waits:
                    extra = waits[:-max_waits]
                    si.on_wait = waits[-max_waits:]
                    for j in range(0, len(extra), max_waits):
                        nop = mybir.InstNoOp(
                            name=f"waitsplit_{n}", ins=[], outs=[],
                            engine=ins.engine)
                        n += 1
                        nop.sync_info = mybir.SyncInfo(
                            on_wait=extra[j:j + max_waits], on_update=[])
                        out.append(nop)
                out.append(ins)
            bb.instructions[:] = out


def _build_program():
    import contextlib

    import concourse.bass as bass
    import concourse.tile as tile
    from concourse import mybir
    from concourse.masks import make_identity

    f32 = mybir.dt.float32
    bf16 = mybir.dt.bfloat16

    nc = bass.Bass("TRN2", target_bir_lowering=False, debug=False,
                   num_devices=NCORES, dynamic_dma_scratch_size=2048)

    xqT_d = nc.dram_tensor("xqT", [D, 8 * P], bf16, kind="ExternalInput").ap()
    xkT_d = nc.dram_tensor("xkT", [D, N], bf16, kind="ExternalInput").ap()
    xkN_d = nc.dram_tensor("xkN", [N, D], bf16, kind="ExternalInput").ap()
    g_d = nc.dram_tensor("g", [D, D], bf16, kind="ExternalInput").ap()
    wv_d = nc.dram_tensor("wv", [D, D], bf16, kind="ExternalInput").ap()
    mask_d = nc.dram_tensor("mask", [P, 2 * P], f32, kind="ExternalInput").ap()
    out_d = nc.dram_tensor("out", [8 * P, D], f32, kind="ExternalOutput").ap()

    xqT_r = xqT_d.rearrange("(dc p) q -> p dc q", p=P)
    xkT_r = xkT_d.rearrange("(dc p) k -> p dc k", p=P)
    xkN_r = xkN_d.rearrange("(kb p) d -> p kb d", p=P)
    g_r = g_d.rearrange("(dc p) e -> p dc e", p=P)
    wv_r = wv_d.rearrange("(dc p) e -> p dc e", p=P)

    with tile.TileContext(nc) as tc:
        with contextlib.ExitStack() as ctx:
            cpool = ctx.enter_context(tc.tile_pool(name="cpool", bufs=1))
            xkp = ctx.enter_context(tc.tile_pool(name="xkp", bufs=1))
            xnp = ctx.enter_context(tc.tile_pool(name="xnp", bufs=1))
            ptp = ctx.enter_context(tc.tile_pool(name="ptp", bufs=1))
            wvp = ctx.enter_context(tc.tile_pool(name="wvp", bufs=1))
            gp = ctx.enter_context(tc.tile_pool(name="gp", bufs=1))
            xqp = ctx.enter_context(tc.tile_pool(name="xqp", bufs=2))
            exp_ = ctx.enter_context(tc.tile_pool(name="exp", bufs=1))
            tbp = ctx.enter_context(tc.tile_pool(name="tbp", bufs=2))
            ttp = ctx.enter_context(tc.tile_pool(name="ttp", bufs=2))
            obp = ctx.enter_context(tc.tile_pool(name="obp", bufs=4))
            rcp = ctx.enter_context(tc.tile_pool(name="rcp", bufs=4))

            # ---- resident tiles ----
            mask_sb = cpool.tile([P, 2 * P], f32, name="mask_sb")
            ones_sb = cpool.tile([P, 1], bf16, name="ones_sb")
            ident_f = cpool.tile([P, P], f32, name="ident_f")
            ident = cpool.tile([P, P], bf16, name="ident")
            XK = xkp.tile([P, DC, N], bf16, name="XK")
            XN = xnp.tile([P, NKB, D], bf16, name="XN")
            PT = ptp.tile([P, DC, 8 * P], bf16, name="PT")
            EX = exp_.tile([P, NKB, 2 * P], bf16, name="EX")

            make_identity(nc, ident_f)
            nc.vector.tensor_copy(ident[:], ident_f[:])
            nc.gpsimd.memset(ones_sb[:], 1.0)

            # ---- input DMAs spread over four queues.  Startup critical
            # path: G quarter 0 + first xq half-chunk (PT's first matmul).
            wvq = [wvp.tile([P, 2, D], bf16, name=f"wv_q{i}")
                   for i in range(4)]
            gq = [gp.tile([P, 2, D], bf16, name=f"g_q{i}") for i in range(4)]

            xqh = [xqp.tile([P, DC, 512], bf16, tag="xq", name=f"xq_h{h}")
                   for h in range(2)]

            def dma_xq(h, half):
                nc.sync.dma_start(
                    xqh[h][:, 4 * half:4 * half + 4, :],
                    xqT_r[:, 4 * half:4 * half + 4, h * 512:(h + 1) * 512])

            nc.sync.dma_start(gq[0][:, 0, :], g_r[:, 0, :])
            nc.sync.dma_start(gq[0][:, 1, :], g_r[:, 1, :])
            dma_xq(0, 0)
            nc.sync.dma_start(gq[1][:], g_r[:, 2:4, :])
            nc.sync.dma_start(gq[2][:], g_r[:, 4:6, :])
            nc.sync.dma_start(gq[3][:], g_r[:, 6:8, :])
            dma_xq(0, 1)
            dma_xq(1, 0)
            dma_xq(1, 1)
            for cchunk in range(4):
                nc.sync.dma_start(
                    XK[:, :, cchunk * 512:(cchunk + 1) * 512],
                    xkT_r[:, :, cchunk * 512:(cchunk + 1) * 512])
            for kq in range(4):
                nc.sync.dma_start(
                    XN[:, 4 * kq:4 * kq + 4, :],
                    xkN_r[:, 4 * kq:4 * kq + 4, :])
            for i in range(4):
                nc.sync.dma_start(wvq[i][:], wv_r[:, 2 * i:2 * i + 2, :])
            nc.sync.dma_start(mask_sb[:], mask_d)

            # ---- PSUM pools are static carve-outs (8 banks total):
            # psc 2 + pav 4 + psm 2.  PT shares pav; the epilogue's
            # transpose bounce shares psc.
            psc = ctx.enter_context(
                tc.tile_pool(name="psc", bufs=2, space="PSUM"))
            pav = ctx.enter_context(
                tc.tile_pool(name="pav", bufs=4, space="PSUM"))
            psm = ctx.enter_context(
                tc.tile_pool(name="psm", bufs=2, space="PSUM"))

            # ---- PT[d, q] = sum_d' G[d',d] xqT[d',q] ----
            if True:
                for qh in range(2):
                    for dct in range(DC):
                        ps = pav.tile([P, 512], f32, tag="pav",
                                      name=f"pspt{qh}_{dct}")
                        for dpc in range(DC):
                            nc.tensor.matmul(
                                ps,
                                gq[dpc // 2][:, dpc % 2,
                                             dct * P:(dct + 1) * P],
                                xqh[qh][:, dpc, :],
                                start=(dpc == 0), stop=(dpc == DC - 1))
                        nc.vector.tensor_copy(
                            PT[:, dct, qh * 512:(qh + 1) * 512], ps)

            # ---- attention, kb-major per slot pair ----
            # Per pair: full scores block -> previous pair's final GEMM
            # (covers its DMA-transpose latency) -> T-accumulation block
            # -> epilogue head (recips, TB casts, DMA-transpose).  The
            # last pair runs an inline PE-transpose epilogue instead so
            # the kernel tail stays short.
            if True:
                epi = {}  # pair -> (tts, rcs, obs-to-emit) state

                def emit_scores(pair, kb):
                    lo, hi = pair
                    both = kb < CAPS[lo]
                    smin = lo if both else hi
                    w = 2 * P if both else P
                    ps = psc.tile([P, 512], f32, tag="psc",
                                  name=f"sc{lo}_{kb}")
                    for dc in range(DC):
                        nc.tensor.matmul(
                            ps[:, :w],
                            XK[:, dc, kb * P:(kb + 1) * P],
                            PT[:, dc, smin * P:smin * P + w],
                            start=(dc == 0), stop=(dc == DC - 1))
                    for s in ((lo, hi) if both else (hi,)):
                        off = (s - smin) * P
                        if kb == CAPS[s] - 2:
                            nc.vector.tensor_add(
                                ps[:, off:off + P], ps[:, off:off + P],
                                mask_sb[:, 0:P])
                        elif kb == CAPS[s] - 1:
                            nc.vector.tensor_add(
                                ps[:, off:off + P], ps[:, off:off + P],
                                mask_sb[:, P:2 * P])
                    nc.scalar.activation(
                        EX[:, kb, 0:w], ps[:, :w],
                        mybir.ActivationFunctionType.Exp)

                def emit_acc(pair, kb, tps, sums):
                    lo, hi = pair
                    both = kb < CAPS[lo]
                    smin = lo if both else hi
                    for s in ((lo, hi) if both else (hi,)):
                        if kb == 0:
                            tps[s] = [pav.tile([P, 512], f32, tag="pav",
                                               name=f"t{s}_{h}")
                                      for h in range(2)]
                            sums[s] = psm.tile([P, 1], f32, tag="psm",
                                               name=f"sums{s}")
                        exs = EX[:, kb, (s - smin) * P:(s - smin + 1) * P]
                        st = (kb == 0)
                        sp = (kb == CAPS[s] - 1)
                        nc.tensor.matmul(sums[s], exs, ones_sb[:],
                                         start=st, stop=sp)
                        for h in range(2):
                            nc.tensor.matmul(
                                tps[s][h], exs,
                                XN[:, kb, h * 512:(h + 1) * 512],
                                start=st, stop=sp)

                def emit_epi_head(pair, tps, sums):
                    """recips, TB casts (1/rowsum folded), DMA-transpose."""
                    lo, hi = pair
                    tts = {}
                    for s in (lo, hi):
                        rc = rcp.tile([P, 1], f32, tag="rc", name=f"rc{s}")
                        nc.vector.reciprocal(rc[:], sums[s])
                        tb = tbp.tile([P, DC, P], bf16, tag="tb",
                                      name=f"tb{s}")
                        for h in range(2):
                            nc.vector.tensor_scalar_mul(
                                tb[:, 4 * h:4 * h + 4, :], tps[s][h], rc[:])
                        tt = ttp.tile([P, DC, P], bf16, tag="tt",
                                      name=f"tt{s}")
                        nc.sync.dma_start_transpose(
                            tt[:], tb[:].rearrange("p a b -> p (a b)"))
                        tts[s] = tt
                    epi[pair] = tts

                def emit_final(pair):
                    lo, hi = pair
                    tts = epi.pop(pair)
                    for s in (lo, hi):
                        ob = obp.tile([P, D], f32, tag="ob", name=f"ob{s}")
                        ops = [pav.tile([P, 512], f32, tag="pav",
                                        name=f"o{s}_{h}") for h in range(2)]
                        for dc in range(DC):
                            for h in range(2):
                                nc.tensor.matmul(
                                    ops[h],
                                    tts[s][:, dc, :],
                                    wvq[dc // 2][:, dc % 2,
                                                 h * 512:(h + 1) * 512],
                                    start=(dc == 0), stop=(dc == DC - 1))
                        nc.vector.tensor_copy(ob[:, 0:512], ops[0])
                        nc.sync.dma_start(
                            out_d[s * P:(s + 1) * P, 0:512], ob[:, 0:512])
                        nc.vector.tensor_copy(ob[:, 512:1024], ops[1])
                        nc.sync.dma_start(
                            out_d[s * P:(s + 1) * P, 512:1024],
                            ob[:, 512:1024])

                def emit_epilogue_inline(pair, tps, sums):
                    """PE-transpose epilogue for the tail pair."""
                    lo, hi = pair
                    rc = {}
                    for s in (lo, hi):
                        rc[s] = rcp.tile([P, 1], f32, tag="rc", name=f"rc{s}")
                        nc.vector.reciprocal(rc[s][:], sums[s])
                    for s in (lo, hi):
                        tb = tbp.tile([P, DC, P], bf16, tag="tb",
                                      name=f"tb{s}")
                        tt = ttp.tile([P, DC, P], bf16, tag="tt",
                                      name=f"tt{s}")
                        ob = obp.tile([P, D], f32, tag="ob", name=f"ob{s}")
                        ops = [pav.tile([P, 512], f32, tag="pav",
                                        name=f"o{s}_{h}") for h in range(2)]
                        for dc in range(DC):
                            src = tps[s][dc // 4][:, (dc % 4) * P:
                                                  (dc % 4 + 1) * P]
                            nc.scalar.activation(
                                tb[:, dc, :], src,
                                mybir.ActivationFunctionType.Copy,
                                scale=rc[s][:])
                            tr = psm.tile([P, P], bf16, tag="psm",
                                          name=f"tr{s}_{dc}")
                            nc.tensor.transpose(tr[:], tb[:, dc, :], ident)
                            nc.vector.tensor_copy(tt[:, dc, :], tr[:])
                            for h in range(2):
                                nc.tensor.matmul(
                                    ops[h],
                                    tt[:, dc, :],
                                    wvq[dc // 2][:, dc % 2,
                                                 h * 512:(h + 1) * 512],
                                    start=(dc == 0), stop=(dc == DC - 1))
                        nc.vector.tensor_copy(ob[:, 0:512], ops[0])
                        nc.sync.dma_start(
                            out_d[s * P:(s + 1) * P, 0:512], ob[:, 0:512])
                        nc.vector.tensor_copy(ob[:, 512:1024], ops[1])
                        nc.sync.dma_start(
                            out_d[s * P:(s + 1) * P, 512:1024],
                            ob[:, 512:1024])

                prev = None
                for pi, pair in enumerate(PAIRS):
                    capmax = CAPS[pair[1]]
                    last = pi == len(PAIRS) - 1
                    tps = {}
                    sums = {}
                    for kb in range(capmax):
                        emit_scores(pair, kb)
                    if prev is not None:
                        emit_final(prev)
                    for kb in range(capmax):
                        emit_acc(pair, kb, tps, sums)
                    if last:
                        emit_epilogue_inline(pair, tps, sums)
                    else:
                        emit_epi_head(pair, tps, sums)
                        prev = pair

    _split_multi_waits(nc)
    return nc


def _host_prep(x, Wq, Wk, Wv):
    """Build per-core input maps."""
    import ml_dtypes

    bf16 = ml_dtypes.bfloat16
    x = np.ascontiguousarray(x, dtype=np.float32)
    G = (np.ascontiguousarray(Wq, np.float32)
         @ np.ascontiguousarray(Wk, np.float32).T) / 32.0
    g_bf = G.astype(bf16)
    wv_bf = np.ascontiguousarray(Wv, np.float32).astype(bf16)

    ki = np.arange(P)[:, None]
    qi = np.arange(P)[None, :]
    tri = np.where(ki <= qi, 0.0, NEG).astype(np.float32)  # [k, q]
    mask_even = np.concatenate(  # diag block, then fully-masked block
        [tri, np.full((P, P), NEG, np.float32)], axis=1)
    mask_odd = np.concatenate(  # fully-visible block, then diag block
        [np.zeros((P, P), np.float32), tri], axis=1)

    in_maps = []
    for c in range(NCORES):
        bi, r = c // 2, c % 2
        qbs = [cap - 2 + r for cap in CAPS]
        xq = np.concatenate(
            [x[bi, qb * P:(qb + 1) * P, :] for qb in qbs], axis=0)
        in_maps.append({
            "xqT": np.ascontiguousarray(xq.T).astype(bf16),
            "xkT": np.ascontiguousarray(x[bi].T).astype(bf16),
            "xkN": x[bi].astype(bf16),
            "g": g_bf,
            "wv": wv_bf,
            "mask": mask_odd if r else mask_even,
        })
    return in_maps


def _host_gather(results):
    out = np.empty((B, N, D), dtype=np.float32)
    for c in range(NCORES):
        bi, r = c // 2, c % 2
        res = results[c]["out"]
        for s, cap in enumerate(CAPS):
            qb = cap - 2 + r
            out[bi, qb * P:(qb + 1) * P, :] = res[s * P:(s + 1) * P, :]
    return out


def kernel(x, Wq, Wk, Wv, _trace=False, _trace_kwargs=None):
    from concourse.bass_utils import run_bass_kernel_spmd

    if "prog" not in _prog_cache:
        _prog_cache["prog"] = _build_program()
    nc = _prog_cache["prog"]

    in_maps = _host_prep(x, Wq, Wk, Wv)
    kw = dict(_trace_kwargs or {})
    res = run_bass_kernel_spmd(nc, in_maps, list(range(NCORES)),
                               trace=_trace, **kw)
    out = _host_gather(res.results)
    if _trace:
        return out, res
    return out
